# revision 21
# baseline (speedup 1.0000x reference)
"""Trainium2 Bass kernel for DQN-LSTM (encoder MLP+LN -> LSTM cell -> decoder MLP).

Contract: kernel(**inputs) takes the FULL unsharded inputs (B=65536) and
returns the full outputs (q, h_n, c_n) exactly like the reference.

Strategy (pure data parallel, 8 cores x 8192 rows):
  - Activations FEATURE-major on chip ([feat, batch]); the host pre-transposes
    x/h/c and post-transposes the outputs, so every matmul is a natural PE op
    (weights stationary, batch on the moving free dim), no on-chip transposes.
  - Matmul operands bf16 (fp32 PSUM accumulation): fp32 matmuls lower to two
    MATMUL passes on trn2, bf16 needs one.  The c-state path, LN statistics,
    broadcast, and the small decoder stay fp32.
  - LayerNorm mean-centering folded into host-precomputed weights
    (W_hat = C @ W, C = I - 11^T/128) so matmul outputs are pre-centered.
    Variance = ones-matmul over squared activations (partition reduction on
    PE); rsqrt = exp(-0.5*ln(var+eps)) on ACT (ACT Rsqrt is blocked for
    accuracy; Ln+Exp share one table set).  Stats of 4 consecutive tiles land
    on PSUM partition strips {0,32,64,96} of one bank via matmul
    tile_position, so a single ACT ln/exp pair serves 4 tiles.
  - ACT table sets pinned: pass A natural_log_exp_and_others, pass B
    sigmoid_and_others -> exactly 2 ACT_TABLE_LOADs per launch.
"""

import os
import sys

sys.path.insert(0, "/opt/trn_rl_repo")
os.environ.setdefault(
    "NEURON_COMPILE_CACHE_URL", os.path.expanduser("~/.neuron_kernel_cache")
)

import numpy as np

B = 65536
NCORES = 8
BS = B // NCORES          # 8192 rows per core
IN_DIM = 28
HID = 128
OUT = 4
EPS = 1e-5
TILE = 512                # batch columns per tile (one PSUM bank of fp32)
GRP = 4                   # tiles per LN-stats group (PSUM strips 0/32/64/96)

_cache = {}


def _build_nc(bs=BS, tile_n=TILE):
    import concourse.bacc as bacc
    import concourse.tile as tile
    from concourse import mybir

    f32 = mybir.dt.float32
    bf16 = mybir.dt.bfloat16
    AF = mybir.ActivationFunctionType
    OP = mybir.AluOpType
    nt = bs // tile_n
    assert nt % GRP == 0

    # Constrain the ACT table-set chooser: the default picker alternates sets
    # per function and emitted 65 ACT_TABLE_LOADs (~83us).  Keep list
    # order/length identical to act_info.json (index = set id); empty every
    # set except the two we want.
    from concourse import hw_specs
    _real_tables = hw_specs.get_activation_tables
    _keep = {
        "natural_log_exp_and_others": {AF.Ln, AF.Exp, AF.Square, AF.Relu},
        "sigmoid_and_others": {AF.Sigmoid, AF.Tanh},
    }

    def _guided_tables(arch):
        full = _real_tables(arch)
        out = {}
        for name, fns in full.items():
            want = _keep.get(name, set())
            assert want <= fns, (name, want - fns)
            out[name] = want
        return out

    bacc.get_activation_tables = _guided_tables

    nc = bacc.Bacc("TRN2", target_bir_lowering=False, debug=False,
                   num_devices=NCORES)

    def din(name, shape, dt=f32):
        return nc.dram_tensor(name, shape, dt, kind="ExternalInput").ap()

    def dout(name, shape):
        return nc.dram_tensor(name, shape, f32, kind="ExternalOutput").ap()

    xT = din("xT", [IN_DIM + 1, bs], bf16)    # [x^T ; ones]
    h0T = din("h0T", [HID, bs], bf16)         # only feeds the W_hh matmul
    c0T = din("c0T", [HID, bs])               # fp32 (c_n arithmetic)
    lhs1_d = din("lhs1", [IN_DIM + 1, HID], bf16)  # [(C@We1)^T ; (C@be1)^T]
    lhs2_d = din("lhs2", [HID, HID], bf16)         # (C@We2)^T
    b2c_d = din("b2c", [HID, 1])                   # (C@be2) column
    g1_d = din("g1c", [HID, 1])
    bt1_d = din("bt1c", [HID, 1])
    g2_d = din("g2c", [HID, 1])
    bt2_d = din("bt2c", [HID, 1])
    wih_d = din("wihT", [HID, 4 * HID], bf16)  # W_ih[perm]^T, order i,f,o,g
    whh_d = din("whhT", [HID, 4 * HID], bf16)
    gb_d = din("gbias", [HID, 4])              # (b_ih+b_hh)[perm], 4 columns
    wd1_d = din("wd1T", [HID, 64])
    bd1_d = din("bd1c", [64, 1])
    wd2_d = din("wd2T", [64, OUT])
    bd2_d = din("bd2c", [OUT, 1])

    qT = dout("qT", [OUT, bs])
    hnT = dout("hnT", [HID, bs])
    cnT = dout("cnT", [HID, bs])

    with tile.TileContext(nc) as tc:
        from contextlib import ExitStack
        ctx = ExitStack()
        with ctx:
            consts = ctx.enter_context(tc.tile_pool(name="consts", bufs=1))
            resid = ctx.enter_context(tc.tile_pool(name="resid", bufs=1))

            _qs = [nc.sync, nc.scalar]
            _qi = [0]

            def cload(dram_ap, shape, tag, dt=f32):
                t = consts.tile(shape, dt, tag=tag)
                _qs[_qi[0] % 2].dma_start(out=t, in_=dram_ap)
                _qi[0] += 1
                return t

            lhs1 = cload(lhs1_d, [IN_DIM + 1, HID], "lhs1", bf16)
            lhs2 = cload(lhs2_d, [HID, HID], "lhs2", bf16)
            b2c = cload(b2c_d, [HID, 1], "b2c")
            g1 = cload(g1_d, [HID, 1], "g1")
            bt1 = cload(bt1_d, [HID, 1], "bt1")
            g2 = cload(g2_d, [HID, 1], "g2")
            bt2 = cload(bt2_d, [HID, 1], "bt2")
            wih = cload(wih_d, [HID, 4 * HID], "wih", bf16)
            whh = cload(whh_d, [HID, 4 * HID], "whh", bf16)
            gb = cload(gb_d, [HID, 4], "gb")
            wd1 = cload(wd1_d, [HID, 64], "wd1")
            bd1 = cload(bd1_d, [64, 1], "bd1")
            wd2 = cload(wd2_d, [64, OUT], "wd2")
            bd2 = cload(bd2_d, [OUT, 1], "bd2")

            # [128,32] ones: the sumsq matmul writes a full 32-row strip of
            # duplicate sums, so the shared stats bank is fully initialized
            ones_k = consts.tile([HID, 32], bf16)
            nc.vector.memset(ones_k, 1.0)
            # rows 0/32/64/96 serve as the K=1 broadcast lhsT at each legal
            # PE row strip (tile_position trick)
            ones_f = consts.tile([HID, HID], bf16)
            nc.vector.memset(ones_f, 1.0)
            epsf = consts.tile([HID, 1], f32)      # LN epsilon bias column
            nc.vector.memset(epsf, EPS)

            z2r = resid.tile([HID, bs], bf16)   # LN2+relu output (gates rhs)

            # ---- pass A: encoder (mm1 -> LN1 -> relu -> mm2 -> LN2 -> relu)
            # ACT funcs: Ln, Exp, Relu - one table set.
            with tc.tile_pool(name="ps1", bufs=3, space="PSUM") as ps1p, \
                 tc.tile_pool(name="pst", bufs=2, space="PSUM") as pstp, \
                 tc.tile_pool(name="ps2", bufs=2, space="PSUM") as ps2p, \
                 tc.tile_pool(name="psi", bufs=1, space="PSUM") as psip, \
                 tc.tile_pool(name="wk", bufs=6) as wk, \
                 tc.tile_pool(name="y1p", bufs=2 * GRP + 2) as y1p, \
                 tc.tile_pool(name="xsp", bufs=2) as xsp, \
                 tc.tile_pool(name="ivp", bufs=3) as ivp:
                for g in range(nt // GRP):
                    xsg = xsp.tile([IN_DIM + 1, GRP * tile_n], bf16,
                                   tag="xsg")
                    nc.sync.dma_start(
                        out=xsg,
                        in_=xT[:, g * GRP * tile_n:(g + 1) * GRP * tile_n])
                    st1 = pstp.tile([HID, tile_n], f32, tag="st1")
                    st2 = pstp.tile([HID, tile_n], f32, tag="st1")
                    y1ts = []
                    for k in range(GRP):
                        ps1 = ps1p.tile([HID, tile_n], f32, tag="ps1")
                        nc.tensor.matmul(ps1, lhs1,
                                         xsg[:, k * tile_n:(k + 1) * tile_n],
                                         start=True, stop=True)
                        y1t = y1p.tile([HID, tile_n], f32, tag="y1t")
                        nc.vector.tensor_copy(y1t, ps1)
                        y1ts.append(y1t)
                        ysq = wk.tile([HID, tile_n], bf16, tag="ysq")
                        nc.gpsimd.tensor_tensor(ysq, y1t, y1t, op=OP.mult)
                        nc.tensor.matmul(st1[32 * k:32 * (k + 1), :], ones_k,
                                         ysq, start=True, stop=True,
                                         tile_position=(0, 32 * k))
                    # rsqrt(var+eps) = exp(-0.5*ln(var+eps)); one ln/exp pair
                    # covers the group's 4 stats strips (other lanes compute
                    # garbage that is never read)
                    lv1 = wk.tile([HID, tile_n], f32, tag="lv1")
                    nc.scalar.activation(lv1, st1, AF.Ln,
                                         bias=epsf, scale=1.0 / HID)
                    iv1 = ivp.tile([HID, tile_n], bf16, tag="iv1")
                    nc.scalar.activation(iv1, lv1, AF.Exp, scale=-0.5)
                    y2ts = []
                    for k in range(GRP):
                        t = g * GRP + k
                        # K=1 bf16 matmul broadcasts the inv row across
                        # partitions (strip 32k via tile_position)
                        piv = psip.tile([HID, tile_n], f32, tag="piv1")
                        nc.tensor.matmul(piv, ones_f[32 * k:32 * k + 1, :],
                                         iv1[32 * k:32 * k + 1, :],
                                         start=True, stop=True,
                                         tile_position=(32 * k, 0))
                        t1 = wk.tile([HID, tile_n], f32, tag="t1")
                        nc.vector.tensor_tensor(t1, y1ts[k], piv, op=OP.mult)
                        z1 = wk.tile([HID, tile_n], bf16, tag="z1")
                        nc.scalar.activation(z1, t1, AF.Relu,
                                             bias=bt1, scale=g1)
                        ps2 = ps2p.tile([HID, tile_n], f32, tag="ps2")
                        nc.tensor.matmul(ps2, lhs2, z1, start=True, stop=True)
                        y2t = y1p.tile([HID, tile_n], f32, tag="y2t")
                        nc.vector.tensor_scalar(y2t, ps2, b2c, None,
                                                op0=OP.add)
                        y2ts.append(y2t)
                        ysq2 = wk.tile([HID, tile_n], bf16, tag="ysq2")
                        nc.gpsimd.tensor_tensor(ysq2, y2t, y2t, op=OP.mult)
                        nc.tensor.matmul(st2[32 * k:32 * (k + 1), :], ones_k,
                                         ysq2, start=True, stop=True,
                                         tile_position=(0, 32 * k))
                    lv2 = wk.tile([HID, tile_n], f32, tag="lv2")
                    nc.scalar.activation(lv2, st2, AF.Ln,
                                         bias=epsf, scale=1.0 / HID)
                    iv2 = ivp.tile([HID, tile_n], bf16, tag="iv2")
                    nc.scalar.activation(iv2, lv2, AF.Exp, scale=-0.5)
                    for k in range(GRP):
                        t = g * GRP + k
                        sl = slice(t * tile_n, (t + 1) * tile_n)
                        piv = psip.tile([HID, tile_n], f32, tag="piv1")
                        nc.tensor.matmul(piv, ones_f[32 * k:32 * k + 1, :],
                                         iv2[32 * k:32 * k + 1, :],
                                         start=True, stop=True,
                                         tile_position=(32 * k, 0))
                        t2 = wk.tile([HID, tile_n], f32, tag="t2")
                        nc.vector.tensor_tensor(t2, y2ts[k], piv, op=OP.mult)
                        nc.scalar.activation(z2r[:, sl], t2, AF.Relu,
                                             bias=bt2, scale=g2)

            # ------- pass B: LSTM cell + decoder (sigmoid set) -------------
            with tc.tile_pool(name="psg", bufs=1, space="PSUM") as psgp, \
                 tc.tile_pool(name="psd", bufs=2, space="PSUM") as psdp, \
                 tc.tile_pool(name="psq", bufs=2, space="PSUM") as psqp, \
                 tc.tile_pool(name="hc", bufs=3) as hcp, \
                 tc.tile_pool(name="ga", bufs=3) as gap, \
                 tc.tile_pool(name="ew", bufs=4) as ewp:
                for t in range(nt):
                    sl = slice(t * tile_n, (t + 1) * tile_n)
                    h0s = hcp.tile([HID, tile_n], bf16, tag="h0s")
                    nc.sync.dma_start(out=h0s, in_=h0T[:, sl])
                    c0s = hcp.tile([HID, tile_n], f32, tag="c0s")
                    nc.sync.dma_start(out=c0s, in_=c0T[:, sl])

                    gts = []
                    for j, tag in enumerate(("gi", "gf", "go", "gg")):
                        wsl = slice(j * HID, (j + 1) * HID)
                        pg = psgp.tile([HID, tile_n], f32, tag="p" + tag)
                        nc.tensor.matmul(pg, wih[:, wsl], z2r[:, sl],
                                         start=True, stop=False)
                        nc.tensor.matmul(pg, whh[:, wsl], h0s,
                                         start=False, stop=True)
                        ga = gap.tile([HID, tile_n], f32, tag=tag)
                        fn = AF.Tanh if tag == "gg" else AF.Sigmoid
                        nc.scalar.activation(ga, pg, fn,
                                             bias=gb[:, j:j + 1])
                        gts.append(ga)
                    gi, gf, go, gg = gts

                    ig = ewp.tile([HID, tile_n], f32, tag="ig")
                    nc.gpsimd.tensor_tensor(ig, gi, gg, op=OP.mult)
                    fc = ewp.tile([HID, tile_n], f32, tag="fc")
                    nc.vector.tensor_tensor(fc, gf, c0s, op=OP.mult)
                    cn = ewp.tile([HID, tile_n], f32, tag="cn")
                    nc.vector.tensor_tensor(cn, fc, ig, op=OP.add)
                    nc.sync.dma_start(out=cnT[:, sl], in_=cn)
                    th = ewp.tile([HID, tile_n], f32, tag="th")
                    nc.scalar.activation(th, cn, AF.Tanh)
                    hn = ewp.tile([HID, tile_n], f32, tag="hn")
                    nc.gpsimd.tensor_tensor(hn, go, th, op=OP.mult)
                    nc.sync.dma_start(out=hnT[:, sl], in_=hn)

                    pd1 = psdp.tile([64, tile_n], f32, tag="pd1")
                    nc.tensor.matmul(pd1, wd1, hn, start=True, stop=True)
                    d1 = ewp.tile([64, tile_n], f32, tag="d1")
                    nc.vector.tensor_scalar(d1, pd1, bd1, 0.0,
                                            op0=OP.add, op1=OP.max)
                    pq = psqp.tile([OUT, tile_n], f32, tag="pq")
                    nc.tensor.matmul(pq, wd2, d1, start=True, stop=True)
                    qs = ewp.tile([OUT, tile_n], f32, tag="qs")
                    nc.vector.tensor_scalar(qs, pq, bd2, None, op0=OP.add)
                    nc.sync.dma_start(out=qT[:, sl], in_=qs)

    nc.compile()
    return nc


def _prep_consts(We1, be1, g1, bt1, We2, be2, g2, bt2,
                 W_ih, W_hh, b_ih, b_hh, Wd1, bd1, Wd2, bd2):
    import ml_dtypes
    bf = ml_dtypes.bfloat16
    f64 = np.float64
    Cm = np.eye(HID, dtype=f64) - 1.0 / HID
    lhs1 = np.concatenate(
        [(Cm @ We1.astype(f64)).T, (Cm @ be1.astype(f64))[None, :]], axis=0
    ).astype(bf)
    lhs2 = np.ascontiguousarray((Cm @ We2.astype(f64)).T).astype(bf)
    b2c = (Cm @ be2.astype(f64))[:, None].astype(np.float32)
    perm = np.r_[0:HID, HID:2 * HID, 3 * HID:4 * HID, 2 * HID:3 * HID]
    wihT = np.ascontiguousarray(W_ih[perm].T).astype(bf)
    whhT = np.ascontiguousarray(W_hh[perm].T).astype(bf)
    gbias = np.ascontiguousarray(
        (b_ih + b_hh)[perm].reshape(4, HID).T.astype(np.float32))
    return {
        "lhs1": lhs1, "lhs2": lhs2, "b2c": b2c,
        "g1c": np.ascontiguousarray(g1[:, None].astype(np.float32)),
        "bt1c": np.ascontiguousarray(bt1[:, None].astype(np.float32)),
        "g2c": np.ascontiguousarray(g2[:, None].astype(np.float32)),
        "bt2c": np.ascontiguousarray(bt2[:, None].astype(np.float32)),
        "wihT": wihT, "whhT": whhT, "gbias": gbias,
        "wd1T": np.ascontiguousarray(Wd1.T.astype(np.float32)),
        "bd1c": np.ascontiguousarray(bd1[:, None].astype(np.float32)),
        "wd2T": np.ascontiguousarray(Wd2.T.astype(np.float32)),
        "bd2c": np.ascontiguousarray(bd2[:, None].astype(np.float32)),
    }


def _make_in_maps(x, h, c, consts):
    import ml_dtypes
    bf = ml_dtypes.bfloat16
    h0 = np.asarray(h[0], dtype=np.float32)
    c0 = np.asarray(c[0], dtype=np.float32)
    x = np.asarray(x, dtype=np.float32)
    ones = np.ones((1, BS), dtype=bf)
    in_maps = []
    for i in range(NCORES):
        sl = slice(i * BS, (i + 1) * BS)
        xTa = np.concatenate(
            [np.ascontiguousarray(x[sl].T).astype(bf), ones], axis=0)
        m = dict(consts)
        m["xT"] = xTa
        m["h0T"] = np.ascontiguousarray(h0[sl].T).astype(bf)
        m["c0T"] = np.ascontiguousarray(c0[sl].T)
        in_maps.append(m)
    return in_maps


def run_on_hw(in_maps, trace=False):
    import time
    from concourse import bass_utils
    if "nc" not in _cache:
        _cache["nc"] = _build_nc()
    nc = _cache["nc"]
    last = None
    # The axon-tunneled devices occasionally come up wedged from a prior
    # session (NRT_EXEC_UNIT_UNRECOVERABLE on the first execute); a retry
    # on a fresh execute recovers.
    for attempt in range(3):
        try:
            return bass_utils.run_bass_kernel_spmd(
                nc, in_maps, core_ids=list(range(NCORES)), trace=trace)
        except Exception as e:  # noqa: BLE001
            last = e
            try:
                import jax
                jax.clear_backends()
            except Exception:
                pass
            time.sleep(5)
    raise last


def kernel(x, h, c, We1, be1, g1, bt1, We2, be2, g2, bt2,
           W_ih, W_hh, b_ih, b_hh, Wd1, bd1, Wd2, bd2):
    consts = _prep_consts(
        np.asarray(We1, np.float32), np.asarray(be1, np.float32),
        np.asarray(g1, np.float32), np.asarray(bt1, np.float32),
        np.asarray(We2, np.float32), np.asarray(be2, np.float32),
        np.asarray(g2, np.float32), np.asarray(bt2, np.float32),
        np.asarray(W_ih, np.float32), np.asarray(W_hh, np.float32),
        np.asarray(b_ih, np.float32), np.asarray(b_hh, np.float32),
        np.asarray(Wd1, np.float32), np.asarray(bd1, np.float32),
        np.asarray(Wd2, np.float32), np.asarray(bd2, np.float32))
    in_maps = _make_in_maps(x, h, c, consts)
    res = run_on_hw(in_maps)
    q = np.concatenate([r["qT"].T for r in res.results], axis=0)
    h_n = np.concatenate([r["hnT"].T for r in res.results], axis=0)[None]
    c_n = np.concatenate([r["cnT"].T for r in res.results], axis=0)[None]
    return (np.ascontiguousarray(q), np.ascontiguousarray(h_n),
            np.ascontiguousarray(c_n))


# revision 22
# speedup vs baseline: 1.1077x; 1.1077x over previous
"""Trainium2 Bass kernel for DQN-LSTM (encoder MLP+LN -> LSTM cell -> decoder MLP).

Contract: kernel(**inputs) takes the FULL unsharded inputs (B=65536) and
returns the full outputs (q, h_n, c_n) exactly like the reference.

Strategy (pure data parallel, 8 cores x 8192 rows):
  - Activations FEATURE-major on chip ([feat, batch]); the host pre-transposes
    x/h/c and post-transposes the outputs, so every matmul is a natural PE op
    (weights stationary, batch on the moving free dim), no on-chip transposes.
  - Matmul operands bf16 (fp32 PSUM accumulation): fp32 matmuls lower to two
    MATMUL passes on trn2, bf16 needs one.  The c-state path, LN statistics,
    broadcast, and the small decoder stay fp32.
  - LayerNorm mean-centering folded into host-precomputed weights
    (W_hat = C @ W, C = I - 11^T/128) so matmul outputs are pre-centered.
    Variance = ones-matmul over squared activations (partition reduction on
    PE); rsqrt = exp(-0.5*ln(var+eps)) on ACT (ACT Rsqrt is blocked for
    accuracy; Ln+Exp share one table set).  Stats of 4 consecutive tiles land
    on PSUM partition strips {0,32,64,96} of one bank via matmul
    tile_position, so a single ACT ln/exp pair serves 4 tiles.
  - ACT table sets pinned: pass A natural_log_exp_and_others, pass B
    sigmoid_and_others -> exactly 2 ACT_TABLE_LOADs per launch.
"""

import os
import sys

sys.path.insert(0, "/opt/trn_rl_repo")
os.environ.setdefault(
    "NEURON_COMPILE_CACHE_URL", os.path.expanduser("~/.neuron_kernel_cache")
)

import numpy as np

B = 65536
NCORES = 8
BS = B // NCORES          # 8192 rows per core
IN_DIM = 28
HID = 128
OUT = 4
EPS = 1e-5
TILE = 512                # batch columns per tile (one PSUM bank of fp32)
GRP = 4                   # tiles per LN-stats group (PSUM strips 0/32/64/96)

_cache = {}


def _build_nc(bs=BS, tile_n=TILE):
    import concourse.bacc as bacc
    import concourse.tile as tile
    from concourse import mybir

    f32 = mybir.dt.float32
    bf16 = mybir.dt.bfloat16
    AF = mybir.ActivationFunctionType
    OP = mybir.AluOpType
    nt = bs // tile_n
    assert nt % GRP == 0

    # Constrain the ACT table-set chooser: the default picker alternates sets
    # per function and emitted 65 ACT_TABLE_LOADs (~83us).  Keep list
    # order/length identical to act_info.json (index = set id); empty every
    # set except the two we want.
    from concourse import hw_specs
    _real_tables = hw_specs.get_activation_tables
    _keep = {
        "natural_log_exp_and_others": {AF.Ln, AF.Exp, AF.Square, AF.Relu},
        "sigmoid_and_others": {AF.Sigmoid, AF.Tanh},
    }

    def _guided_tables(arch):
        full = _real_tables(arch)
        out = {}
        for name, fns in full.items():
            want = _keep.get(name, set())
            assert want <= fns, (name, want - fns)
            out[name] = want
        return out

    bacc.get_activation_tables = _guided_tables

    nc = bacc.Bacc("TRN2", target_bir_lowering=False, debug=False,
                   num_devices=NCORES)

    def din(name, shape, dt=f32):
        return nc.dram_tensor(name, shape, dt, kind="ExternalInput").ap()

    def dout(name, shape):
        return nc.dram_tensor(name, shape, f32, kind="ExternalOutput").ap()

    xT = din("xT", [IN_DIM + 1, bs], bf16)    # [x^T ; ones]
    h0T = din("h0T", [HID, bs], bf16)         # only feeds the W_hh matmul
    c0T = din("c0T", [HID, bs])               # fp32 (c_n arithmetic)
    lhs1_d = din("lhs1", [IN_DIM + 1, HID], bf16)  # [(C@We1)^T ; (C@be1)^T]
    lhs2_d = din("lhs2", [HID, HID], bf16)         # (C@We2)^T
    b2c_d = din("b2c", [HID, 1])                   # (C@be2) column
    g1_d = din("g1c", [HID, 1])
    bt1_d = din("bt1c", [HID, 1])
    g2_d = din("g2c", [HID, 1])
    bt2_d = din("bt2c", [HID, 1])
    wih_d = din("wihT", [HID, 4 * HID], bf16)  # W_ih[perm]^T, order i,f,o,g
    whh_d = din("whhT", [HID, 4 * HID], bf16)
    gb_d = din("gbias", [HID, 4])              # (b_ih+b_hh)[perm], 4 columns
    wd1_d = din("wd1T", [HID, 64])
    bd1_d = din("bd1c", [64, 1])
    wd2_d = din("wd2T", [64, OUT])
    bd2_d = din("bd2c", [OUT, 1])

    qT = dout("qT", [OUT, bs])
    hnT = dout("hnT", [HID, bs])
    cnT = dout("cnT", [HID, bs])

    with tile.TileContext(nc) as tc:
        from contextlib import ExitStack
        ctx = ExitStack()
        with ctx:
            consts = ctx.enter_context(tc.tile_pool(name="consts", bufs=1))
            resid = ctx.enter_context(tc.tile_pool(name="resid", bufs=1))

            _qs = [nc.sync, nc.scalar]
            _qi = [0]

            def cload(dram_ap, shape, tag, dt=f32):
                t = consts.tile(shape, dt, tag=tag)
                _qs[_qi[0] % 2].dma_start(out=t, in_=dram_ap)
                _qi[0] += 1
                return t

            lhs1 = cload(lhs1_d, [IN_DIM + 1, HID], "lhs1", bf16)
            lhs2 = cload(lhs2_d, [HID, HID], "lhs2", bf16)
            b2c = cload(b2c_d, [HID, 1], "b2c")
            g1 = cload(g1_d, [HID, 1], "g1")
            bt1 = cload(bt1_d, [HID, 1], "bt1")
            g2 = cload(g2_d, [HID, 1], "g2")
            bt2 = cload(bt2_d, [HID, 1], "bt2")
            wih = cload(wih_d, [HID, 4 * HID], "wih", bf16)
            whh = cload(whh_d, [HID, 4 * HID], "whh", bf16)
            gb = cload(gb_d, [HID, 4], "gb")
            wd1 = cload(wd1_d, [HID, 64], "wd1")
            bd1 = cload(bd1_d, [64, 1], "bd1")
            wd2 = cload(wd2_d, [64, OUT], "wd2")
            bd2 = cload(bd2_d, [OUT, 1], "bd2")

            # [128,32] ones: the sumsq matmul writes a full 32-row strip of
            # duplicate sums, so the shared stats bank is fully initialized
            ones_k = consts.tile([HID, 32], bf16)
            nc.vector.memset(ones_k, 1.0)
            # rows 0/32/64/96 serve as the K=1 broadcast lhsT at each legal
            # PE row strip (tile_position trick)
            ones_f = consts.tile([HID, HID], bf16)
            nc.vector.memset(ones_f, 1.0)
            epsf = consts.tile([HID, 1], f32)      # LN epsilon bias column
            nc.vector.memset(epsf, EPS)

            z2r = resid.tile([HID, bs], bf16)   # LN2+relu output (gates rhs)

            # ---- pass A: encoder (mm1 -> LN1 -> relu -> mm2 -> LN2 -> relu)
            # ACT funcs: Ln, Exp, Relu - one table set.
            with tc.tile_pool(name="ps1", bufs=2, space="PSUM") as ps1p, \
                 tc.tile_pool(name="pst", bufs=1, space="PSUM") as pstp, \
                 tc.tile_pool(name="ps2", bufs=2, space="PSUM") as ps2p, \
                 tc.tile_pool(name="psi", bufs=1, space="PSUM") as psip, \
                 tc.tile_pool(name="wk", bufs=4) as wk, \
                 tc.tile_pool(name="y1p", bufs=2 * GRP) as y1p, \
                 tc.tile_pool(name="xsp", bufs=2) as xsp, \
                 tc.tile_pool(name="ivp", bufs=2) as ivp:
                for g in range(nt // GRP):
                    xsg = xsp.tile([IN_DIM + 1, GRP * tile_n], bf16,
                                   tag="xsg")
                    nc.sync.dma_start(
                        out=xsg,
                        in_=xT[:, g * GRP * tile_n:(g + 1) * GRP * tile_n])
                    st1 = pstp.tile([HID, tile_n], f32, tag="st1")
                    st2 = pstp.tile([HID, tile_n], f32, tag="st2")
                    y1ts = []
                    for k in range(GRP):
                        ps1 = ps1p.tile([HID, tile_n], f32, tag="ps1")
                        nc.tensor.matmul(ps1, lhs1,
                                         xsg[:, k * tile_n:(k + 1) * tile_n],
                                         start=True, stop=True)
                        y1t = y1p.tile([HID, tile_n], f32, tag="y1t")
                        nc.vector.tensor_copy(y1t, ps1)
                        y1ts.append(y1t)
                        ysq = wk.tile([HID, tile_n], bf16, tag="ysq")
                        nc.gpsimd.tensor_tensor(ysq, y1t, y1t, op=OP.mult)
                        nc.tensor.matmul(st1[32 * k:32 * (k + 1), :], ones_k,
                                         ysq, start=True, stop=True,
                                         tile_position=(0, 32 * k))
                    # rsqrt(var+eps) = exp(-0.5*ln(var+eps)); one ln/exp pair
                    # covers the group's 4 stats strips (other lanes compute
                    # garbage that is never read)
                    lv1 = wk.tile([HID, tile_n], f32, tag="lv1")
                    nc.scalar.activation(lv1, st1, AF.Ln,
                                         bias=epsf, scale=1.0 / HID)
                    iv1 = ivp.tile([HID, tile_n], bf16, tag="iv1")
                    nc.scalar.activation(iv1, lv1, AF.Exp, scale=-0.5)
                    y2ts = []
                    for k in range(GRP):
                        t = g * GRP + k
                        # K=1 bf16 matmul broadcasts the inv row across
                        # partitions (strip 32k via tile_position)
                        piv = psip.tile([HID, tile_n], f32, tag="piv1")
                        nc.tensor.matmul(piv, ones_f[32 * k:32 * k + 1, :],
                                         iv1[32 * k:32 * k + 1, :],
                                         start=True, stop=True,
                                         tile_position=(32 * k, 0))
                        t1 = wk.tile([HID, tile_n], f32, tag="t1")
                        nc.vector.tensor_tensor(t1, y1ts[k], piv, op=OP.mult)
                        z1 = wk.tile([HID, tile_n], bf16, tag="z1")
                        nc.scalar.activation(z1, t1, AF.Relu,
                                             bias=bt1, scale=g1)
                        ps2 = ps2p.tile([HID, tile_n], f32, tag="ps2")
                        nc.tensor.matmul(ps2, lhs2, z1, start=True, stop=True)
                        y2t = y1p.tile([HID, tile_n], f32, tag="y2t")
                        nc.vector.tensor_scalar(y2t, ps2, b2c, None,
                                                op0=OP.add)
                        y2ts.append(y2t)
                        ysq2 = wk.tile([HID, tile_n], bf16, tag="ysq2")
                        nc.gpsimd.tensor_tensor(ysq2, y2t, y2t, op=OP.mult)
                        nc.tensor.matmul(st2[32 * k:32 * (k + 1), :], ones_k,
                                         ysq2, start=True, stop=True,
                                         tile_position=(0, 32 * k))
                    lv2 = wk.tile([HID, tile_n], f32, tag="lv2")
                    nc.scalar.activation(lv2, st2, AF.Ln,
                                         bias=epsf, scale=1.0 / HID)
                    iv2 = ivp.tile([HID, tile_n], bf16, tag="iv2")
                    nc.scalar.activation(iv2, lv2, AF.Exp, scale=-0.5)
                    for k in range(GRP):
                        t = g * GRP + k
                        sl = slice(t * tile_n, (t + 1) * tile_n)
                        piv = psip.tile([HID, tile_n], f32, tag="piv2")
                        nc.tensor.matmul(piv, ones_f[32 * k:32 * k + 1, :],
                                         iv2[32 * k:32 * k + 1, :],
                                         start=True, stop=True,
                                         tile_position=(32 * k, 0))
                        t2 = wk.tile([HID, tile_n], f32, tag="t2")
                        nc.vector.tensor_tensor(t2, y2ts[k], piv, op=OP.mult)
                        nc.scalar.activation(z2r[:, sl], t2, AF.Relu,
                                             bias=bt2, scale=g2)

            # ------- pass B: LSTM cell + decoder (sigmoid set) -------------
            with tc.tile_pool(name="psg", bufs=1, space="PSUM") as psgp, \
                 tc.tile_pool(name="psd", bufs=2, space="PSUM") as psdp, \
                 tc.tile_pool(name="psq", bufs=2, space="PSUM") as psqp, \
                 tc.tile_pool(name="hc", bufs=3) as hcp, \
                 tc.tile_pool(name="ga", bufs=3) as gap, \
                 tc.tile_pool(name="ew", bufs=4) as ewp:
                for t in range(nt):
                    sl = slice(t * tile_n, (t + 1) * tile_n)
                    h0s = hcp.tile([HID, tile_n], bf16, tag="h0s")
                    nc.sync.dma_start(out=h0s, in_=h0T[:, sl])
                    c0s = hcp.tile([HID, tile_n], f32, tag="c0s")
                    nc.sync.dma_start(out=c0s, in_=c0T[:, sl])

                    gts = []
                    for j, tag in enumerate(("gi", "gf", "go", "gg")):
                        wsl = slice(j * HID, (j + 1) * HID)
                        pg = psgp.tile([HID, tile_n], f32, tag="p" + tag)
                        nc.tensor.matmul(pg, wih[:, wsl], z2r[:, sl],
                                         start=True, stop=False)
                        nc.tensor.matmul(pg, whh[:, wsl], h0s,
                                         start=False, stop=True)
                        ga = gap.tile([HID, tile_n], f32, tag=tag)
                        fn = AF.Tanh if tag == "gg" else AF.Sigmoid
                        nc.scalar.activation(ga, pg, fn,
                                             bias=gb[:, j:j + 1])
                        gts.append(ga)
                    gi, gf, go, gg = gts

                    ig = ewp.tile([HID, tile_n], f32, tag="ig")
                    nc.gpsimd.tensor_tensor(ig, gi, gg, op=OP.mult)
                    fc = ewp.tile([HID, tile_n], f32, tag="fc")
                    nc.vector.tensor_tensor(fc, gf, c0s, op=OP.mult)
                    cn = ewp.tile([HID, tile_n], f32, tag="cn")
                    nc.vector.tensor_tensor(cn, fc, ig, op=OP.add)
                    nc.sync.dma_start(out=cnT[:, sl], in_=cn)
                    th = ewp.tile([HID, tile_n], f32, tag="th")
                    nc.scalar.activation(th, cn, AF.Tanh)
                    hn = ewp.tile([HID, tile_n], f32, tag="hn")
                    nc.gpsimd.tensor_tensor(hn, go, th, op=OP.mult)
                    nc.sync.dma_start(out=hnT[:, sl], in_=hn)

                    pd1 = psdp.tile([64, tile_n], f32, tag="pd1")
                    nc.tensor.matmul(pd1, wd1, hn, start=True, stop=True)
                    d1 = ewp.tile([64, tile_n], f32, tag="d1")
                    nc.vector.tensor_scalar(d1, pd1, bd1, 0.0,
                                            op0=OP.add, op1=OP.max)
                    pq = psqp.tile([OUT, tile_n], f32, tag="pq")
                    nc.tensor.matmul(pq, wd2, d1, start=True, stop=True)
                    qs = ewp.tile([OUT, tile_n], f32, tag="qs")
                    nc.vector.tensor_scalar(qs, pq, bd2, None, op0=OP.add)
                    nc.sync.dma_start(out=qT[:, sl], in_=qs)

    nc.compile()
    return nc


def _prep_consts(We1, be1, g1, bt1, We2, be2, g2, bt2,
                 W_ih, W_hh, b_ih, b_hh, Wd1, bd1, Wd2, bd2):
    import ml_dtypes
    bf = ml_dtypes.bfloat16
    f64 = np.float64
    Cm = np.eye(HID, dtype=f64) - 1.0 / HID
    lhs1 = np.concatenate(
        [(Cm @ We1.astype(f64)).T, (Cm @ be1.astype(f64))[None, :]], axis=0
    ).astype(bf)
    lhs2 = np.ascontiguousarray((Cm @ We2.astype(f64)).T).astype(bf)
    b2c = (Cm @ be2.astype(f64))[:, None].astype(np.float32)
    perm = np.r_[0:HID, HID:2 * HID, 3 * HID:4 * HID, 2 * HID:3 * HID]
    wihT = np.ascontiguousarray(W_ih[perm].T).astype(bf)
    whhT = np.ascontiguousarray(W_hh[perm].T).astype(bf)
    gbias = np.ascontiguousarray(
        (b_ih + b_hh)[perm].reshape(4, HID).T.astype(np.float32))
    return {
        "lhs1": lhs1, "lhs2": lhs2, "b2c": b2c,
        "g1c": np.ascontiguousarray(g1[:, None].astype(np.float32)),
        "bt1c": np.ascontiguousarray(bt1[:, None].astype(np.float32)),
        "g2c": np.ascontiguousarray(g2[:, None].astype(np.float32)),
        "bt2c": np.ascontiguousarray(bt2[:, None].astype(np.float32)),
        "wihT": wihT, "whhT": whhT, "gbias": gbias,
        "wd1T": np.ascontiguousarray(Wd1.T.astype(np.float32)),
        "bd1c": np.ascontiguousarray(bd1[:, None].astype(np.float32)),
        "wd2T": np.ascontiguousarray(Wd2.T.astype(np.float32)),
        "bd2c": np.ascontiguousarray(bd2[:, None].astype(np.float32)),
    }


def _make_in_maps(x, h, c, consts):
    import ml_dtypes
    bf = ml_dtypes.bfloat16
    h0 = np.asarray(h[0], dtype=np.float32)
    c0 = np.asarray(c[0], dtype=np.float32)
    x = np.asarray(x, dtype=np.float32)
    ones = np.ones((1, BS), dtype=bf)
    in_maps = []
    for i in range(NCORES):
        sl = slice(i * BS, (i + 1) * BS)
        xTa = np.concatenate(
            [np.ascontiguousarray(x[sl].T).astype(bf), ones], axis=0)
        m = dict(consts)
        m["xT"] = xTa
        m["h0T"] = np.ascontiguousarray(h0[sl].T).astype(bf)
        m["c0T"] = np.ascontiguousarray(c0[sl].T)
        in_maps.append(m)
    return in_maps


def run_on_hw(in_maps, trace=False):
    import time
    from concourse import bass_utils
    if "nc" not in _cache:
        _cache["nc"] = _build_nc()
    nc = _cache["nc"]
    last = None
    # The axon-tunneled devices occasionally come up wedged from a prior
    # session (NRT_EXEC_UNIT_UNRECOVERABLE on the first execute); a retry
    # on a fresh execute recovers.
    for attempt in range(3):
        try:
            return bass_utils.run_bass_kernel_spmd(
                nc, in_maps, core_ids=list(range(NCORES)), trace=trace)
        except Exception as e:  # noqa: BLE001
            last = e
            try:
                import jax
                jax.clear_backends()
            except Exception:
                pass
            time.sleep(5)
    raise last


def kernel(x, h, c, We1, be1, g1, bt1, We2, be2, g2, bt2,
           W_ih, W_hh, b_ih, b_hh, Wd1, bd1, Wd2, bd2):
    consts = _prep_consts(
        np.asarray(We1, np.float32), np.asarray(be1, np.float32),
        np.asarray(g1, np.float32), np.asarray(bt1, np.float32),
        np.asarray(We2, np.float32), np.asarray(be2, np.float32),
        np.asarray(g2, np.float32), np.asarray(bt2, np.float32),
        np.asarray(W_ih, np.float32), np.asarray(W_hh, np.float32),
        np.asarray(b_ih, np.float32), np.asarray(b_hh, np.float32),
        np.asarray(Wd1, np.float32), np.asarray(bd1, np.float32),
        np.asarray(Wd2, np.float32), np.asarray(bd2, np.float32))
    in_maps = _make_in_maps(x, h, c, consts)
    res = run_on_hw(in_maps)
    q = np.concatenate([r["qT"].T for r in res.results], axis=0)
    h_n = np.concatenate([r["hnT"].T for r in res.results], axis=0)[None]
    c_n = np.concatenate([r["cnT"].T for r in res.results], axis=0)[None]
    return (np.ascontiguousarray(q), np.ascontiguousarray(h_n),
            np.ascontiguousarray(c_n))


# revision 23
# speedup vs baseline: 1.1078x; 1.0000x over previous
"""Trainium2 Bass kernel for DQN-LSTM (encoder MLP+LN -> LSTM cell -> decoder MLP).

Contract: kernel(**inputs) takes the FULL unsharded inputs (B=65536) and
returns the full outputs (q, h_n, c_n) exactly like the reference.

Strategy (pure data parallel, 8 cores x 8192 rows):
  - Activations FEATURE-major on chip ([feat, batch]); the host pre-transposes
    x/h/c and post-transposes the outputs, so every matmul is a natural PE op
    (weights stationary, batch on the moving free dim), no on-chip transposes.
  - Matmul operands bf16 (fp32 PSUM accumulation): fp32 matmuls lower to two
    MATMUL passes on trn2, bf16 needs one.  The c-state path, LN statistics,
    broadcast, and the small decoder stay fp32.
  - LayerNorm mean-centering folded into host-precomputed weights
    (W_hat = C @ W, C = I - 11^T/128) so matmul outputs are pre-centered.
    Variance = ones-matmul over squared activations (partition reduction on
    PE); rsqrt = exp(-0.5*ln(var+eps)) on ACT (ACT Rsqrt is blocked for
    accuracy; Ln+Exp share one table set).  Stats of 4 consecutive tiles land
    on PSUM partition strips {0,32,64,96} of one bank via matmul
    tile_position, so a single ACT ln/exp pair serves 4 tiles.
  - ACT table sets pinned: pass A natural_log_exp_and_others, pass B
    sigmoid_and_others -> exactly 2 ACT_TABLE_LOADs per launch.
"""

import os
import sys

sys.path.insert(0, "/opt/trn_rl_repo")
os.environ.setdefault(
    "NEURON_COMPILE_CACHE_URL", os.path.expanduser("~/.neuron_kernel_cache")
)

import numpy as np

B = 65536
NCORES = 8
BS = B // NCORES          # 8192 rows per core
IN_DIM = 28
HID = 128
OUT = 4
EPS = 1e-5
TILE = 512                # batch columns per tile (one PSUM bank of fp32)
GRP = 4                   # tiles per LN-stats group (PSUM strips 0/32/64/96)

_cache = {}


def _build_nc(bs=BS, tile_n=TILE):
    import concourse.bacc as bacc
    import concourse.tile as tile
    from concourse import mybir

    f32 = mybir.dt.float32
    bf16 = mybir.dt.bfloat16
    AF = mybir.ActivationFunctionType
    OP = mybir.AluOpType
    nt = bs // tile_n
    assert nt % GRP == 0

    # Constrain the ACT table-set chooser: the default picker alternates sets
    # per function and emitted 65 ACT_TABLE_LOADs (~83us).  Keep list
    # order/length identical to act_info.json (index = set id); empty every
    # set except the two we want.
    from concourse import hw_specs
    _real_tables = hw_specs.get_activation_tables
    _keep = {
        "natural_log_exp_and_others": {AF.Ln, AF.Exp, AF.Square, AF.Relu},
        "sigmoid_and_others": {AF.Sigmoid, AF.Tanh},
    }

    def _guided_tables(arch):
        full = _real_tables(arch)
        out = {}
        for name, fns in full.items():
            want = _keep.get(name, set())
            assert want <= fns, (name, want - fns)
            out[name] = want
        return out

    bacc.get_activation_tables = _guided_tables

    nc = bacc.Bacc("TRN2", target_bir_lowering=False, debug=False,
                   num_devices=NCORES)

    def din(name, shape, dt=f32):
        return nc.dram_tensor(name, shape, dt, kind="ExternalInput").ap()

    def dout(name, shape):
        return nc.dram_tensor(name, shape, f32, kind="ExternalOutput").ap()

    xT = din("xT", [IN_DIM + 1, bs], bf16)    # [x^T ; ones]
    h0T = din("h0T", [HID, bs], bf16)         # only feeds the W_hh matmul
    c0T = din("c0T", [HID, bs])               # fp32 (c_n arithmetic)
    lhs1_d = din("lhs1", [IN_DIM + 1, HID], bf16)  # [(C@We1)^T ; (C@be1)^T]
    lhs2_d = din("lhs2", [HID, HID], bf16)         # (C@We2)^T
    b2c_d = din("b2c", [HID, 1])                   # (C@be2) column
    g1_d = din("g1c", [HID, 1])
    bt1_d = din("bt1c", [HID, 1])
    g2_d = din("g2c", [HID, 1])
    bt2_d = din("bt2c", [HID, 1])
    wih_d = din("wihT", [HID, 4 * HID], bf16)  # W_ih[perm]^T, order i,f,o,g
    whh_d = din("whhT", [HID, 4 * HID], bf16)
    gb_d = din("gbias", [HID, 4])              # (b_ih+b_hh)[perm], 4 columns
    wd1_d = din("wd1T", [HID, 64])
    bd1_d = din("bd1c", [64, 1])
    wd2_d = din("wd2T", [64, OUT])
    bd2_d = din("bd2c", [OUT, 1])

    qT = dout("qT", [OUT, bs])
    hnT = dout("hnT", [HID, bs])
    cnT = dout("cnT", [HID, bs])

    with tile.TileContext(nc) as tc:
        from contextlib import ExitStack
        ctx = ExitStack()
        with ctx:
            consts = ctx.enter_context(tc.tile_pool(name="consts", bufs=1))
            resid = ctx.enter_context(tc.tile_pool(name="resid", bufs=1))

            _qs = [nc.sync, nc.scalar]
            _qi = [0]

            def cload(dram_ap, shape, tag, dt=f32):
                t = consts.tile(shape, dt, tag=tag)
                _qs[_qi[0] % 2].dma_start(out=t, in_=dram_ap)
                _qi[0] += 1
                return t

            lhs1 = cload(lhs1_d, [IN_DIM + 1, HID], "lhs1", bf16)
            lhs2 = cload(lhs2_d, [HID, HID], "lhs2", bf16)
            b2c = cload(b2c_d, [HID, 1], "b2c")
            g1 = cload(g1_d, [HID, 1], "g1")
            bt1 = cload(bt1_d, [HID, 1], "bt1")
            g2 = cload(g2_d, [HID, 1], "g2")
            bt2 = cload(bt2_d, [HID, 1], "bt2")
            wih = cload(wih_d, [HID, 4 * HID], "wih", bf16)
            whh = cload(whh_d, [HID, 4 * HID], "whh", bf16)
            gb = cload(gb_d, [HID, 4], "gb")
            wd1 = cload(wd1_d, [HID, 64], "wd1")
            bd1 = cload(bd1_d, [64, 1], "bd1")
            wd2 = cload(wd2_d, [64, OUT], "wd2")
            bd2 = cload(bd2_d, [OUT, 1], "bd2")

            # [128,32] ones: the sumsq matmul writes a full 32-row strip of
            # duplicate sums, so the shared stats bank is fully initialized
            ones_k = consts.tile([HID, 32], bf16)
            nc.vector.memset(ones_k, 1.0)
            # rows 0/32/64/96 serve as the K=1 broadcast lhsT at each legal
            # PE row strip (tile_position trick)
            ones_f = consts.tile([HID, HID], bf16)
            nc.vector.memset(ones_f, 1.0)
            epsf = consts.tile([HID, 1], f32)      # LN epsilon bias column
            nc.vector.memset(epsf, EPS)

            z2r = resid.tile([HID, bs], bf16)   # LN2+relu output (gates rhs)

            # ---- pass A: encoder (mm1 -> LN1 -> relu -> mm2 -> LN2 -> relu)
            # ACT funcs: Ln, Exp, Relu - one table set.
            with tc.tile_pool(name="ps1", bufs=2, space="PSUM") as ps1p, \
                 tc.tile_pool(name="pst", bufs=1, space="PSUM") as pstp, \
                 tc.tile_pool(name="ps2", bufs=2, space="PSUM") as ps2p, \
                 tc.tile_pool(name="psi", bufs=1, space="PSUM") as psip, \
                 tc.tile_pool(name="wk", bufs=5) as wk, \
                 tc.tile_pool(name="y1p", bufs=2 * GRP + 2) as y1p, \
                 tc.tile_pool(name="xsp", bufs=2) as xsp, \
                 tc.tile_pool(name="ivp", bufs=3) as ivp:
                for g in range(nt // GRP):
                    xsg = xsp.tile([IN_DIM + 1, GRP * tile_n], bf16,
                                   tag="xsg")
                    nc.sync.dma_start(
                        out=xsg,
                        in_=xT[:, g * GRP * tile_n:(g + 1) * GRP * tile_n])
                    st1 = pstp.tile([HID, tile_n], f32, tag="st1")
                    st2 = pstp.tile([HID, tile_n], f32, tag="st2")
                    y1ts = []
                    for k in range(GRP):
                        ps1 = ps1p.tile([HID, tile_n], f32, tag="ps1")
                        nc.tensor.matmul(ps1, lhs1,
                                         xsg[:, k * tile_n:(k + 1) * tile_n],
                                         start=True, stop=True)
                        y1t = y1p.tile([HID, tile_n], f32, tag="y1t")
                        nc.vector.tensor_copy(y1t, ps1)
                        y1ts.append(y1t)
                        ysq = wk.tile([HID, tile_n], bf16, tag="ysq")
                        nc.gpsimd.tensor_tensor(ysq, y1t, y1t, op=OP.mult)
                        nc.tensor.matmul(st1[32 * k:32 * (k + 1), :], ones_k,
                                         ysq, start=True, stop=True,
                                         tile_position=(0, 32 * k))
                    # rsqrt(var+eps) = exp(-0.5*ln(var+eps)); one ln/exp pair
                    # covers the group's 4 stats strips (other lanes compute
                    # garbage that is never read)
                    lv1 = wk.tile([HID, tile_n], f32, tag="lv1")
                    nc.scalar.activation(lv1, st1, AF.Ln,
                                         bias=epsf, scale=1.0 / HID)
                    iv1 = ivp.tile([HID, tile_n], bf16, tag="iv1")
                    nc.scalar.activation(iv1, lv1, AF.Exp, scale=-0.5)
                    y2ts = []
                    for k in range(GRP):
                        t = g * GRP + k
                        # K=1 bf16 matmul broadcasts the inv row across
                        # partitions (strip 32k via tile_position)
                        piv = psip.tile([HID, tile_n], f32, tag="piv1")
                        nc.tensor.matmul(piv, ones_f[32 * k:32 * k + 1, :],
                                         iv1[32 * k:32 * k + 1, :],
                                         start=True, stop=True,
                                         tile_position=(32 * k, 0))
                        t1 = wk.tile([HID, tile_n], f32, tag="t1")
                        nc.vector.tensor_tensor(t1, y1ts[k], piv, op=OP.mult)
                        z1 = wk.tile([HID, tile_n], bf16, tag="z1")
                        nc.scalar.activation(z1, t1, AF.Relu,
                                             bias=bt1, scale=g1)
                        ps2 = ps2p.tile([HID, tile_n], f32, tag="ps2")
                        nc.tensor.matmul(ps2, lhs2, z1, start=True, stop=True)
                        y2t = y1p.tile([HID, tile_n], f32, tag="y2t")
                        nc.vector.tensor_scalar(y2t, ps2, b2c, None,
                                                op0=OP.add)
                        y2ts.append(y2t)
                        ysq2 = wk.tile([HID, tile_n], bf16, tag="ysq2")
                        nc.gpsimd.tensor_tensor(ysq2, y2t, y2t, op=OP.mult)
                        nc.tensor.matmul(st2[32 * k:32 * (k + 1), :], ones_k,
                                         ysq2, start=True, stop=True,
                                         tile_position=(0, 32 * k))
                    lv2 = wk.tile([HID, tile_n], f32, tag="lv2")
                    nc.scalar.activation(lv2, st2, AF.Ln,
                                         bias=epsf, scale=1.0 / HID)
                    iv2 = ivp.tile([HID, tile_n], bf16, tag="iv2")
                    nc.scalar.activation(iv2, lv2, AF.Exp, scale=-0.5)
                    for k in range(GRP):
                        t = g * GRP + k
                        sl = slice(t * tile_n, (t + 1) * tile_n)
                        piv = psip.tile([HID, tile_n], f32, tag="piv2")
                        nc.tensor.matmul(piv, ones_f[32 * k:32 * k + 1, :],
                                         iv2[32 * k:32 * k + 1, :],
                                         start=True, stop=True,
                                         tile_position=(32 * k, 0))
                        t2 = wk.tile([HID, tile_n], f32, tag="t2")
                        nc.vector.tensor_tensor(t2, y2ts[k], piv, op=OP.mult)
                        nc.scalar.activation(z2r[:, sl], t2, AF.Relu,
                                             bias=bt2, scale=g2)

            # ------- pass B: LSTM cell + decoder (sigmoid set) -------------
            with tc.tile_pool(name="psg", bufs=1, space="PSUM") as psgp, \
                 tc.tile_pool(name="psd", bufs=2, space="PSUM") as psdp, \
                 tc.tile_pool(name="psq", bufs=2, space="PSUM") as psqp, \
                 tc.tile_pool(name="hc", bufs=4) as hcp, \
                 tc.tile_pool(name="ga", bufs=4) as gap, \
                 tc.tile_pool(name="ew", bufs=5) as ewp:
                for t in range(nt):
                    sl = slice(t * tile_n, (t + 1) * tile_n)
                    h0s = hcp.tile([HID, tile_n], bf16, tag="h0s")
                    nc.sync.dma_start(out=h0s, in_=h0T[:, sl])
                    c0s = hcp.tile([HID, tile_n], f32, tag="c0s")
                    nc.sync.dma_start(out=c0s, in_=c0T[:, sl])

                    gts = []
                    for j, tag in enumerate(("gi", "gf", "go", "gg")):
                        wsl = slice(j * HID, (j + 1) * HID)
                        pg = psgp.tile([HID, tile_n], f32, tag="p" + tag)
                        nc.tensor.matmul(pg, wih[:, wsl], z2r[:, sl],
                                         start=True, stop=False)
                        nc.tensor.matmul(pg, whh[:, wsl], h0s,
                                         start=False, stop=True)
                        ga = gap.tile([HID, tile_n], f32, tag=tag)
                        fn = AF.Tanh if tag == "gg" else AF.Sigmoid
                        nc.scalar.activation(ga, pg, fn,
                                             bias=gb[:, j:j + 1])
                        gts.append(ga)
                    gi, gf, go, gg = gts

                    ig = ewp.tile([HID, tile_n], f32, tag="ig")
                    nc.gpsimd.tensor_tensor(ig, gi, gg, op=OP.mult)
                    fc = ewp.tile([HID, tile_n], f32, tag="fc")
                    nc.vector.tensor_tensor(fc, gf, c0s, op=OP.mult)
                    cn = ewp.tile([HID, tile_n], f32, tag="cn")
                    nc.vector.tensor_tensor(cn, fc, ig, op=OP.add)
                    nc.sync.dma_start(out=cnT[:, sl], in_=cn)
                    th = ewp.tile([HID, tile_n], f32, tag="th")
                    nc.scalar.activation(th, cn, AF.Tanh)
                    hn = ewp.tile([HID, tile_n], f32, tag="hn")
                    nc.gpsimd.tensor_tensor(hn, go, th, op=OP.mult)
                    nc.sync.dma_start(out=hnT[:, sl], in_=hn)

                    pd1 = psdp.tile([64, tile_n], f32, tag="pd1")
                    nc.tensor.matmul(pd1, wd1, hn, start=True, stop=True)
                    d1 = ewp.tile([64, tile_n], f32, tag="d1")
                    nc.vector.tensor_scalar(d1, pd1, bd1, 0.0,
                                            op0=OP.add, op1=OP.max)
                    pq = psqp.tile([OUT, tile_n], f32, tag="pq")
                    nc.tensor.matmul(pq, wd2, d1, start=True, stop=True)
                    qs = ewp.tile([OUT, tile_n], f32, tag="qs")
                    nc.vector.tensor_scalar(qs, pq, bd2, None, op0=OP.add)
                    nc.sync.dma_start(out=qT[:, sl], in_=qs)

    nc.compile()
    return nc


def _prep_consts(We1, be1, g1, bt1, We2, be2, g2, bt2,
                 W_ih, W_hh, b_ih, b_hh, Wd1, bd1, Wd2, bd2):
    import ml_dtypes
    bf = ml_dtypes.bfloat16
    f64 = np.float64
    Cm = np.eye(HID, dtype=f64) - 1.0 / HID
    lhs1 = np.concatenate(
        [(Cm @ We1.astype(f64)).T, (Cm @ be1.astype(f64))[None, :]], axis=0
    ).astype(bf)
    lhs2 = np.ascontiguousarray((Cm @ We2.astype(f64)).T).astype(bf)
    b2c = (Cm @ be2.astype(f64))[:, None].astype(np.float32)
    perm = np.r_[0:HID, HID:2 * HID, 3 * HID:4 * HID, 2 * HID:3 * HID]
    wihT = np.ascontiguousarray(W_ih[perm].T).astype(bf)
    whhT = np.ascontiguousarray(W_hh[perm].T).astype(bf)
    gbias = np.ascontiguousarray(
        (b_ih + b_hh)[perm].reshape(4, HID).T.astype(np.float32))
    return {
        "lhs1": lhs1, "lhs2": lhs2, "b2c": b2c,
        "g1c": np.ascontiguousarray(g1[:, None].astype(np.float32)),
        "bt1c": np.ascontiguousarray(bt1[:, None].astype(np.float32)),
        "g2c": np.ascontiguousarray(g2[:, None].astype(np.float32)),
        "bt2c": np.ascontiguousarray(bt2[:, None].astype(np.float32)),
        "wihT": wihT, "whhT": whhT, "gbias": gbias,
        "wd1T": np.ascontiguousarray(Wd1.T.astype(np.float32)),
        "bd1c": np.ascontiguousarray(bd1[:, None].astype(np.float32)),
        "wd2T": np.ascontiguousarray(Wd2.T.astype(np.float32)),
        "bd2c": np.ascontiguousarray(bd2[:, None].astype(np.float32)),
    }


def _make_in_maps(x, h, c, consts):
    import ml_dtypes
    bf = ml_dtypes.bfloat16
    h0 = np.asarray(h[0], dtype=np.float32)
    c0 = np.asarray(c[0], dtype=np.float32)
    x = np.asarray(x, dtype=np.float32)
    ones = np.ones((1, BS), dtype=bf)
    in_maps = []
    for i in range(NCORES):
        sl = slice(i * BS, (i + 1) * BS)
        xTa = np.concatenate(
            [np.ascontiguousarray(x[sl].T).astype(bf), ones], axis=0)
        m = dict(consts)
        m["xT"] = xTa
        m["h0T"] = np.ascontiguousarray(h0[sl].T).astype(bf)
        m["c0T"] = np.ascontiguousarray(c0[sl].T)
        in_maps.append(m)
    return in_maps


def run_on_hw(in_maps, trace=False):
    import time
    from concourse import bass_utils
    if "nc" not in _cache:
        _cache["nc"] = _build_nc()
    nc = _cache["nc"]
    last = None
    # The axon-tunneled devices occasionally come up wedged from a prior
    # session (NRT_EXEC_UNIT_UNRECOVERABLE on the first execute); a retry
    # on a fresh execute recovers.
    for attempt in range(3):
        try:
            return bass_utils.run_bass_kernel_spmd(
                nc, in_maps, core_ids=list(range(NCORES)), trace=trace)
        except Exception as e:  # noqa: BLE001
            last = e
            try:
                import jax
                jax.clear_backends()
            except Exception:
                pass
            time.sleep(5)
    raise last


def kernel(x, h, c, We1, be1, g1, bt1, We2, be2, g2, bt2,
           W_ih, W_hh, b_ih, b_hh, Wd1, bd1, Wd2, bd2):
    consts = _prep_consts(
        np.asarray(We1, np.float32), np.asarray(be1, np.float32),
        np.asarray(g1, np.float32), np.asarray(bt1, np.float32),
        np.asarray(We2, np.float32), np.asarray(be2, np.float32),
        np.asarray(g2, np.float32), np.asarray(bt2, np.float32),
        np.asarray(W_ih, np.float32), np.asarray(W_hh, np.float32),
        np.asarray(b_ih, np.float32), np.asarray(b_hh, np.float32),
        np.asarray(Wd1, np.float32), np.asarray(bd1, np.float32),
        np.asarray(Wd2, np.float32), np.asarray(bd2, np.float32))
    in_maps = _make_in_maps(x, h, c, consts)
    res = run_on_hw(in_maps)
    q = np.concatenate([r["qT"].T for r in res.results], axis=0)
    h_n = np.concatenate([r["hnT"].T for r in res.results], axis=0)[None]
    c_n = np.concatenate([r["cnT"].T for r in res.results], axis=0)[None]
    return (np.ascontiguousarray(q), np.ascontiguousarray(h_n),
            np.ascontiguousarray(c_n))
